# revision 11
# baseline (speedup 1.0000x reference)
"""DiT block (sparse block-causal diffusion attention) on 8 TRN2 NeuronCores.

Sharding: cores 0-3 -> batch 0, cores 4-7 -> batch 1. Within a 4-core batch
group, attention is tensor-parallel over heads (3 heads/core, zero redundant
matmul work); the attention-output projection produces per-core partials that
a ReduceScatter turns into each core's 512-token slice, on which the core
runs residual + LN2 + MLP and writes its slice of the output.

Attention exploits the mask structure: noised queries attend their own
16-token block plus strictly-earlier clean blocks; clean queries attend
clean blocks inclusively (block-causal). Masking is done inside the PE via
tiny rank-8/9 additive matmuls; softmax denominators come from a ones-column
appended to V. All matmuls run bf16 with fp32 PSUM accumulation; LN/softmax
normalization/residual arithmetic stays fp32.
"""
import os
import sys
from contextlib import ExitStack

for _p in ('/opt/trn_rl_repo', '/root/.axon_site/_ro/trn_rl_repo'):
    if os.path.isdir(_p) and _p not in sys.path:
        sys.path.insert(0, _p)

import numpy as np
import ml_dtypes

import concourse.bass as bass
import concourse.tile as tile
from concourse import bacc, mybir
from concourse.bass_utils import run_bass_kernel_spmd

BF16 = ml_dtypes.bfloat16
FP32 = mybir.dt.float32
BF = mybir.dt.bfloat16

B, N, DIM, H, HD, COND, BS = 2, 1024, 768, 12, 64, 128, 16
S = 2 * N
HID = 4 * DIM
EPS = 1e-5
NEGM = -8192.0            # additive mask value; exact in bf16, exp() -> 0 in fp32
NCORES, GROUP = 8, 4
HPC = H // GROUP          # 3 heads per core
FPC = HPC * HD            # 192 projection cols per core per q/k/v
TOK = S // GROUP          # 512-token output slice per core
PT = 128                  # partition tile
NT = S // PT              # 16 token tiles per batch
NTH = N // PT             # 8 tiles per (noised|clean) half
KD = DIM // PT            # 6 contraction chunks for DIM
KH = HID // PT            # 24 contraction chunks for HID
QKVW = 3 * FPC            # 576
SCALE = 1.0 / float(np.sqrt(HD))

# per-head e^T buffer column layout: [cross_j regions][noised diag][clean_j]
CROSS_OFF = []
_o = 0
for _j in range(NTH):
    CROSS_OFF.append(_o)
    _o += N - PT * _j
CROSS_END = _o                      # 4608
NDIAG_OFF = CROSS_END               # + 128*i
CLEAN_OFF = CROSS_END + N           # 5632; clean regions reuse CROSS_OFF shape
ET_COLS = CLEAN_OFF + CROSS_END     # 10240

LAST_EXEC_NS = None
_CACHE = {}


def _bf(a):
    return np.ascontiguousarray(np.asarray(a, np.float32).astype(BF16))


def _f32(a):
    return np.ascontiguousarray(np.asarray(a, np.float32))


def _hilo(a):
    a = np.asarray(a, np.float32)
    hi = a.astype(BF16)
    lo = (a - hi.astype(np.float32)).astype(BF16)
    return np.ascontiguousarray(np.stack([hi, lo]))


def host_inputs(x, c, cos, sin, mask, norm1_w, qkv_w, attn_out_w, norm2_w,
                mlp_w1, mlp_b1, mlp_w2, mlp_b2, adaLN_w, adaLN_b):
    """Build the 8 per-core input maps (sharding + layout prep, no model math)."""
    x = _f32(x); c = _f32(c)
    qkv_w = _f32(qkv_w); attn_out_w = _f32(attn_out_w)
    cos2 = _f32(cos).reshape(S, HD)
    sin2 = _f32(sin).reshape(S, HD)
    # natural-layout rope tables tiled across the 9 (proj,head) 64-col groups
    cosb = np.tile(cos2, (1, QKVW // HD))                       # [S, 576]
    sgn = np.where(np.arange(HD) < HD // 2, -1.0, 1.0)          # rotate_half sign
    sinb = np.tile(sin2 * sgn[None, :], (1, QKVW // HD))
    cosb = _bf(cosb).reshape(NT, PT, QKVW)
    sinb = _bf(sinb).reshape(NT, PT, QKVW)

    # additive block masks, rank-factored (block index within a 128-token tile)
    blk = np.arange(PT) // BS
    nb = PT // BS                                                # 8
    msl = np.stack([(blk == m).astype(np.float32) for m in range(nb)])
    msr_strict = np.stack([NEGM * (blk <= m) for m in range(nb)])
    msr_incl = np.stack([NEGM * (blk < m) for m in range(nb)])
    mdl = np.zeros((nb + 1, PT), np.float32)
    mdr = np.zeros((nb + 1, PT), np.float32)
    mdl[0] = 1.0
    mdr[0] = NEGM
    for m in range(nb):
        mdl[1 + m] = (blk == m)
        mdr[1 + m] = -NEGM * (blk == m)

    shared = dict(
        cosb=cosb, sinb=sinb,
        msl=_bf(msl), msr_strict=_bf(msr_strict), msr_incl=_bf(msr_incl),
        mdl=_bf(mdl), mdr=_bf(mdr),
        ada_wT=_hilo(adaLN_w.T),
        ada_b=_f32(adaLN_b).reshape(1, 6 * DIM),
        n1w=_f32(norm1_w).reshape(1, DIM), n2w=_f32(norm2_w).reshape(1, DIM),
        w1T=_bf(mlp_w1.T).reshape(KD, PT, HID),
        w2T=_bf(mlp_w2.T).reshape(KH, PT, DIM),
        b1c=_f32(mlp_b1).reshape(KH, PT).T.copy(),
        b2r=_f32(mlp_b2).reshape(1, DIM),
    )

    in_maps = []
    for core in range(NCORES):
        b, g = core // GROUP, core % GROUP
        h0 = HPC * g
        def hc(p, h):
            base = p * DIM + (h0 + h) * HD
            return np.arange(base, base + HD)
        cols = np.concatenate([
            hc(0, 0), hc(0, 1), hc(1, 0), hc(1, 1),
            hc(0, 2), hc(1, 2), hc(2, 0), hc(2, 1), hc(2, 2)])  # qkv_w rows
        m = dict(shared)
        m.update(
            x_b=_f32(x[b]),
            x_own=_f32(x[b, TOK * g:TOK * (g + 1)]),
            cvec=_hilo(c[b].reshape(COND, 1)).reshape(2, COND).T.copy(),
            qkv_wT=_bf(qkv_w[cols].T).reshape(KD, PT, QKVW),
            woutT=_bf(attn_out_w[:, h0 * HD:(h0 + HPC) * HD].T),
        )
        in_maps.append(m)
    return in_maps


def build_program(taps=False):
    nc = bacc.Bacc("TRN2", target_bir_lowering=False, debug=False,
                   num_devices=NCORES)
    D = {}

    def din(name, shape, dt=FP32):
        D[name] = nc.dram_tensor(name, list(shape), dt, kind="ExternalInput").ap()

    def dout(name, shape, dt=FP32):
        D[name] = nc.dram_tensor(name, list(shape), dt, kind="ExternalOutput").ap()

    din('x_b', (S, DIM)); din('x_own', (TOK, DIM)); din('cvec', (COND, 2), BF)
    din('ada_wT', (2, COND, 6 * DIM), BF); din('ada_b', (1, 6 * DIM))
    din('n1w', (1, DIM)); din('n2w', (1, DIM))
    din('qkv_wT', (KD, PT, QKVW), BF)
    din('woutT', (FPC, DIM), BF)
    din('w1T', (KD, PT, HID), BF)
    din('w2T', (KH, PT, DIM), BF)
    din('b1c', (PT, KH)); din('b2r', (1, DIM))
    din('cosb', (NT, PT, QKVW), BF); din('sinb', (NT, PT, QKVW), BF)
    din('msl', (8, PT), BF); din('msr_strict', (8, PT), BF)
    din('msr_incl', (8, PT), BF)
    din('mdl', (9, PT), BF); din('mdr', (9, PT), BF)
    dout('out_slice', (TOK, DIM))
    if taps:
        dout('h_dbg', (S, DIM), BF)
        dout('qkvr_dbg', (S, QKVW), BF)
        dout('aT_dbg', (FPC, S), BF)
        dout('P_dbg', (S, DIM), BF)
        dout('x2_dbg', (TOK, DIM))
        dout('mods_dbg', (1, 6 * DIM), BF)

    with tile.TileContext(nc) as tc, ExitStack() as ctx:
        build_body(ctx, tc, D, taps)
    nc.compile()
    return nc


def build_body(ctx, tc, D, taps):
    nc = tc.nc
    AF = mybir.ActivationFunctionType
    ALU = mybir.AluOpType

    persist = ctx.enter_context(tc.tile_pool(name="persist", bufs=1))
    stream = ctx.enter_context(tc.tile_pool(name="stream", bufs=2))
    xpool = ctx.enter_context(tc.tile_pool(name="xpool", bufs=2))
    hpool = ctx.enter_context(tc.tile_pool(name="hpool", bufs=2))
    vec = ctx.enter_context(tc.tile_pool(name="vec", bufs=1))
    stats = ctx.enter_context(tc.tile_pool(name="stats", bufs=4))
    rcp = ctx.enter_context(tc.tile_pool(name="rcp", bufs=2))
    dummy = ctx.enter_context(tc.tile_pool(name="dummy", bufs=1))
    dram = ctx.enter_context(tc.tile_pool(name="dram", bufs=1, space="DRAM"))
    # PSUM: 8 banks total -> 4 pools x 2 bufs x 1 bank
    pq = ctx.enter_context(tc.tile_pool(name="pq", bufs=2, space="PSUM"))
    psc = ctx.enter_context(tc.tile_pool(name="psc", bufs=2, space="PSUM"))
    ppv = ctx.enter_context(tc.tile_pool(name="ppv", bufs=2, space="PSUM"))
    pmm = ctx.enter_context(tc.tile_pool(name="pmm", bufs=2, space="PSUM"))

    # ---- resident weights/tables --------------------------------------
    qkvw_sb = persist.tile([PT, KD * QKVW], BF, tag="bigA")
    for k in range(KD):
        nc.sync.dma_start(qkvw_sb[:, QKVW * k:QKVW * (k + 1)], D['qkv_wT'][k])
    woutT_sb = persist.tile([PT, 2 * DIM], BF, tag="woutT")
    nc.sync.dma_start(woutT_sb[:, 0:DIM], D['woutT'][0:PT])
    nc.sync.dma_start(woutT_sb[0:FPC - PT, DIM:2 * DIM], D['woutT'][PT:FPC])
    b1_sb = persist.tile([PT, KH], FP32, tag="b1")
    nc.sync.dma_start(b1_sb[:], D['b1c'][:])

    def load_small(name, shape, dt=BF):
        t = persist.tile(list(shape), dt, tag=name)
        nc.sync.dma_start(t[:], D[name][:])
        return t

    msl_sb = load_small('msl', (8, PT))
    msrs_sb = load_small('msr_strict', (8, PT))
    msri_sb = load_small('msr_incl', (8, PT))
    mdl_sb = load_small('mdl', (9, PT))
    mdr_sb = load_small('mdr', (9, PT))

    # ---- adaLN: mods = c @ adaLN_w.T + b; modulation vectors ----------
    cvec_sb = vec.tile([COND, 2], BF, tag="cvec")
    nc.sync.dma_start(cvec_sb[:], D['cvec'][:])
    n1w_sb = vec.tile([1, DIM], FP32, tag="n1w")
    nc.sync.dma_start(n1w_sb[:], D['n1w'][:])
    n2w_sb = vec.tile([1, DIM], FP32, tag="n2w")
    nc.sync.dma_start(n2w_sb[:], D['n2w'][:])
    b2_sb = vec.tile([1, DIM], FP32, tag="b2")
    nc.sync.dma_start(b2_sb[:], D['b2r'][:])

    mods = vec.tile([1, 6 * DIM], BF, tag="mods")
    for i in range(6 * DIM // 512):
        awh = stream.tile([PT, 512], BF, tag="adawh")
        nc.sync.dma_start(awh[:], D['ada_wT'][0][:, 512 * i:512 * (i + 1)])
        awl = stream.tile([PT, 512], BF, tag="adawl")
        nc.sync.dma_start(awl[:], D['ada_wT'][1][:, 512 * i:512 * (i + 1)])
        ab = stream.tile([1, 512], FP32, tag="adab")
        nc.sync.dma_start(ab[:], D['ada_b'][:, 512 * i:512 * (i + 1)])
        pm = pq.tile([PT, 512], FP32, tag="pq")
        nc.tensor.matmul(pm[0:1, :], cvec_sb[:, 0:1], awh[:],
                         start=True, stop=False)
        nc.tensor.matmul(pm[0:1, :], cvec_sb[:, 1:2], awh[:],
                         start=False, stop=False)
        nc.tensor.matmul(pm[0:1, :], cvec_sb[:, 0:1], awl[:],
                         start=False, stop=True)
        nc.vector.tensor_add(mods[:, 512 * i:512 * (i + 1)], pm[0:1, :], ab[:])
    if taps:
        nc.sync.dma_start(D['mods_dbg'][:], mods[:])

    sh1, sc1, g1, sh2, sc2, g2 = (mods[:, DIM * i:DIM * (i + 1)]
                                  for i in range(6))
    sw1 = vec.tile([1, DIM], FP32, tag="sw1")
    nc.vector.tensor_scalar_add(sw1[:], sc1, 1.0)
    nc.vector.tensor_mul(sw1[:], sw1[:], n1w_sb[:])
    sw2 = vec.tile([1, DIM], FP32, tag="sw2")
    nc.vector.tensor_scalar_add(sw2[:], sc2, 1.0)
    nc.vector.tensor_mul(sw2[:], sw2[:], n2w_sb[:])
    gb2 = vec.tile([1, DIM], FP32, tag="gb2")
    nc.vector.tensor_mul(gb2[:], g2, b2_sb[:])

    def bcast(src, name, dt=BF):
        t = persist.tile([PT, DIM], dt, tag=name)
        if src.dtype != dt:
            s2 = vec.tile([1, DIM], dt, tag=name + "_c")
            nc.vector.tensor_copy(s2[:], src)
            src = s2[:]
        nc.gpsimd.partition_broadcast(t[:], src)
        return t

    sw1b = bcast(sw1[:], "sw1b")
    sh1b = bcast(sh1, "sh1b")
    sw2b = bcast(sw2[:], "sw2b")
    sh2b = bcast(sh2, "sh2b")
    g1b = bcast(g1, "g1b")
    g2b = bcast(g2, "g2b")
    gb2b = bcast(gb2[:], "gb2b", FP32)

    # ---- big shared SBUF buffers --------------------------------------
    qkt = persist.tile([PT, 4 * S], BF, tag="bigB")  # [q01 | k01 | q2k2 | k2@0]
    vsb = persist.tile([PT, NT * HPC * (HD + 1)], BF, tag="vsb")
    nc.vector.memset(vsb[:], 1.0)                      # ones col baked in
    aT01 = persist.tile([PT, S], BF, tag="bigC")
    aT2 = persist.tile([HD, S], BF, tag="bigB")

    def vslot(t, h):
        o = (t * HPC + h) * (HD + 1)
        return vsb[:, o:o + HD + 1]

    eps_sb = vec.tile([PT, 1], FP32, tag="eps")
    nc.vector.memset(eps_sb[:], EPS)

    def layernorm_mod(xin, swb, shb, tag):
        """h = (x - mean)/std * swb + shb -> bf16 tile [PT, DIM]."""
        sq = dummy.tile([PT, DIM], BF, tag="sqd")
        ssq = stats.tile([PT, 1], FP32, tag="ssq")
        nc.scalar.activation(sq[:], xin, AF.Square, accum_out=ssq[:])
        sx = stats.tile([PT, 1], FP32, tag="sx")
        nc.vector.reduce_sum(sx[:], xin, axis=mybir.AxisListType.X)
        mu = stats.tile([PT, 1], FP32, tag="mu")
        nc.vector.tensor_scalar_mul(mu[:], sx[:], 1.0 / DIM)
        mu2 = stats.tile([PT, 1], FP32, tag="mu2")
        nc.vector.tensor_mul(mu2[:], mu[:], mu[:])
        var = stats.tile([PT, 1], FP32, tag="var")
        nc.vector.tensor_scalar(var[:], ssq[:], 1.0 / DIM, mu2[:],
                                ALU.mult, ALU.subtract)
        std = stats.tile([PT, 1], FP32, tag="std")
        nc.scalar.activation(std[:], var[:], AF.Sqrt, bias=eps_sb[:])
        rstd = stats.tile([PT, 1], FP32, tag="rstd")
        nc.vector.reciprocal(rstd[:], std[:])
        xcn = hpool.tile([PT, DIM], BF, tag="xcn")
        nc.vector.tensor_scalar(xcn[:], xin, mu[:], rstd[:],
                                ALU.subtract, ALU.mult)
        hm = hpool.tile([PT, DIM], BF, tag="hm")
        nc.gpsimd.tensor_mul(hm[:], xcn[:], swb[:])
        ht = hpool.tile([PT, DIM], BF, tag=tag)
        nc.vector.tensor_add(ht[:], hm[:], shb[:])
        return ht

    # ---- phase 1: LN1 + qkv + rope per 128-token tile -----------------
    for t in range(NT):
        xt = xpool.tile([PT, DIM], FP32, tag="xt")
        nc.sync.dma_start(xt[:], D['x_b'][PT * t:PT * (t + 1)])
        ht = layernorm_mod(xt[:], sw1b, sh1b, "ht")
        if taps:
            nc.sync.dma_start(D['h_dbg'][PT * t:PT * (t + 1)], ht[:])
        hTt = hpool.tile([PT, DIM], BF, tag="hTt")     # h^T chunks for this tile
        for k in range(KD):
            nc.sync.dma_start_transpose(hTt[:, PT * k:PT * (k + 1)],
                                        ht[:, PT * k:PT * (k + 1)])
        qr = hpool.tile([PT, QKVW], BF, tag="qr")
        for nch in range(2):
            n0 = 288 * nch
            pt_ = pq.tile([PT, 512], FP32, tag="pq")
            for k in range(KD):
                nc.tensor.matmul(
                    pt_[:, 0:288], hTt[:, PT * k:PT * (k + 1)],
                    qkvw_sb[:, QKVW * k + n0:QKVW * k + n0 + 288],
                    start=(k == 0), stop=(k == KD - 1))
            if (t + nch) % 2 == 0:
                nc.vector.tensor_copy(qr[:, n0:n0 + 288], pt_[:, 0:288])
            else:
                nc.scalar.copy(qr[:, n0:n0 + 288], pt_[:, 0:288])
        # rope: qkvr = qkv*cos + shifted(qkv)*sin_signed
        cst = stream.tile([PT, QKVW], BF, tag="cosb")
        nc.sync.dma_start(cst[:], D['cosb'][t])
        snt = stream.tile([PT, QKVW], BF, tag="sinb")
        nc.sync.dma_start(snt[:], D['sinb'][t])
        t1 = hpool.tile([PT, QKVW], BF, tag="ropet1")
        nc.vector.tensor_mul(t1[:], qr[:], cst[:])
        t2 = hpool.tile([PT, QKVW], BF, tag="ropet2")
        hh = HD // 2

        def g3(ap_, lohi):
            v = ap_.rearrange("p (g d) -> p g d", d=HD)
            return v[:, :, 0:hh] if lohi == 0 else v[:, :, hh:HD]

        nc.gpsimd.tensor_tensor(g3(t2[:], 0), g3(qr[:], 1), g3(snt[:], 0),
                                ALU.mult)
        nc.gpsimd.tensor_tensor(g3(t2[:], 1), g3(qr[:], 0), g3(snt[:], 1),
                                ALU.mult)
        qkvr = hpool.tile([PT, QKVW], BF, tag="qkvr")
        nc.vector.tensor_add(qkvr[:], t1[:], t2[:])
        if taps:
            nc.sync.dma_start(D['qkvr_dbg'][PT * t:PT * (t + 1)], qkvr[:])
        for r in range(3):   # (q0,q1) (k0,k1) (q2,k2) pair transposes
            nc.sync.dma_start_transpose(
                qkt[:, S * r + PT * t:S * r + PT * (t + 1)],
                qkvr[:, PT * r:PT * (r + 1)])
        nc.sync.dma_start(qkt[0:HD, 3 * S + PT * t:3 * S + PT * (t + 1)],
                          qkt[HD:PT, 2 * S + PT * t:2 * S + PT * (t + 1)])
        for h in range(HPC):
            nc.sync.dma_start(vslot(t, h)[:, 0:HD],
                              qkvr[:, 2 * FPC + HD * h:2 * FPC + HD * (h + 1)])

    # ---- phase 2: attention per head ----------------------------------
    eT = persist.tile([PT, ET_COLS], BF, tag="bigA")

    def qT(h, a, b_):
        if h == 0:
            return qkt[0:HD, a:b_]
        if h == 1:
            return qkt[HD:PT, a:b_]
        return qkt[0:HD, 2 * S + a:2 * S + b_]

    def kT(h, a, b_):
        if h == 0:
            return qkt[0:HD, S + a:S + b_]
        if h == 1:
            return qkt[HD:PT, S + a:S + b_]
        return qkt[0:HD, 3 * S + a:3 * S + b_]

    for h in range(HPC):
        # scores + exp; keys are always clean tiles; two query halves
        for qoff, eoff, msr in ((0, 0, msrs_sb), (N, CLEAN_OFF, msri_sb)):
            for j in range(NTH):
                k0 = N + PT * j
                qlo = qoff + PT * j
                nq = (qoff + N) - qlo
                for c0 in range(0, nq, 512):
                    w = min(512, nq - c0)
                    ps = psc.tile([PT, 512], FP32, tag="psc")
                    has_mask = (c0 == 0)
                    nc.tensor.matmul(ps[:, 0:w], kT(h, k0, k0 + PT),
                                     qT(h, qlo + c0, qlo + c0 + w),
                                     start=True, stop=not has_mask)
                    if has_mask:
                        nc.tensor.matmul(ps[:, 0:PT], msl_sb[:], msr[:],
                                         start=False, stop=True)
                    dst = eoff + CROSS_OFF[j] + c0
                    nc.scalar.activation(eT[:, dst:dst + w], ps[:, 0:w],
                                         AF.Exp, scale=SCALE)
        for i in range(NTH):  # noised diagonal (noised keys)
            k0 = PT * i
            ps = psc.tile([PT, 512], FP32, tag="psc")
            nc.tensor.matmul(ps[:, 0:PT], kT(h, k0, k0 + PT),
                             qT(h, k0, k0 + PT), start=True, stop=False)
            nc.tensor.matmul(ps[:, 0:PT], mdl_sb[:], mdr_sb[:],
                             start=False, stop=True)
            dst = NDIAG_OFF + PT * i
            nc.scalar.activation(eT[:, dst:dst + PT], ps[:, 0:PT],
                                 AF.Exp, scale=SCALE)
        # pv per 512-query group; lhsT = [v | 1] -> row 64 = softmax denom
        for G in range(4):
            noised = G < 2
            Gq = G % 2
            qbase = 512 * Gq                   # within the half
            eoff = 0 if noised else CLEAN_OFF
            jmax = 4 * Gq + 3
            po = ppv.tile([PT, 512], FP32, tag="ppv")
            for j in range(jmax + 1):
                qlo = max(qbase, PT * j)
                w = qbase + 512 - qlo
                src = eoff + CROSS_OFF[j] + (qlo - PT * j)
                last = (j == jmax) and not noised
                nc.tensor.matmul(po[0:HD + 1, 512 - w:512],
                                 vslot(NTH + j, h)[:], eT[:, src:src + w],
                                 start=(j == 0), stop=last)
            if noised:
                for i in range(4 * Gq, 4 * Gq + 4):
                    nc.tensor.matmul(
                        po[0:HD + 1, PT * (i - 4 * Gq):PT * (i - 4 * Gq + 1)],
                        vslot(i, h)[:],
                        eT[:, NDIAG_OFF + PT * i:NDIAG_OFF + PT * (i + 1)],
                        start=False, stop=(i == 4 * Gq + 3))
            rc = rcp.tile([1, 512], FP32, tag="rc")
            nc.vector.reciprocal(rc[:], po[HD:HD + 1, :])
            rb = rcp.tile([HD, 512], FP32, tag="rb")
            nc.gpsimd.partition_broadcast(rb[:], rc[:])
            qg = 512 * G
            if h < 2:
                dst = aT01[HD * h:HD * (h + 1), qg:qg + 512]
            else:
                dst = aT2[:, qg:qg + 512]
            nc.vector.tensor_mul(dst, po[0:HD, :], rb[:])

    if taps:
        nc.sync.dma_start(D['aT_dbg'][0:PT], aT01[:])
        nc.sync.dma_start(D['aT_dbg'][PT:FPC], aT2[:])

    # ---- phase 3: out-projection partials + ReduceScatter --------------
    dP = dram.tile([S, DIM], BF)
    dPrs = dram.tile([TOK, DIM], BF)
    for t in range(NT):
        Psb = hpool.tile([PT, DIM], BF, tag="Psb")
        for n in range(2):
            pp = pmm.tile([PT, 512], FP32, tag="pmm")
            nc.tensor.matmul(pp[:, 0:384], aT01[:, PT * t:PT * (t + 1)],
                             woutT_sb[:, 384 * n:384 * (n + 1)],
                             start=True, stop=False)
            nc.tensor.matmul(pp[:, 0:384], aT2[:, PT * t:PT * (t + 1)],
                             woutT_sb[0:HD, DIM + 384 * n:DIM + 384 * (n + 1)],
                             start=False, stop=True)
            if (t + n) % 2 == 0:
                nc.vector.tensor_copy(Psb[:, 384 * n:384 * (n + 1)],
                                      pp[:, 0:384])
            else:
                nc.scalar.copy(Psb[:, 384 * n:384 * (n + 1)], pp[:, 0:384])
        nc.sync.dma_start(dP[PT * t:PT * (t + 1), :], Psb[:])
        if taps:
            nc.sync.dma_start(D['P_dbg'][PT * t:PT * (t + 1)], Psb[:])
    nc.gpsimd.collective_compute(
        "ReduceScatter", mybir.AluOpType.add,
        replica_groups=[[0, 1, 2, 3], [4, 5, 6, 7]],
        ins=[dP.opt()], outs=[dPrs.opt()])

    # ---- phase 4: x2 = g1*P + x_own; LN2; h2^T ------------------------
    x2 = persist.tile([PT, (TOK // PT) * DIM], FP32, tag="bigC")
    h2T = persist.tile([PT, KD * TOK], BF, tag="bigB")
    for t in range(TOK // PT):
        prs = hpool.tile([PT, DIM], BF, tag="prs")
        nc.sync.dma_start(prs[:], dPrs[PT * t:PT * (t + 1), :])
        xo = xpool.tile([PT, DIM], FP32, tag="xt")
        nc.sync.dma_start(xo[:], D['x_own'][PT * t:PT * (t + 1)])
        gp = hpool.tile([PT, DIM], FP32, tag="gp")
        nc.vector.tensor_mul(gp[:], prs[:], g1b[:])
        x2t = x2[:, DIM * t:DIM * (t + 1)]
        nc.gpsimd.tensor_add(x2t, gp[:], xo[:])
        if taps:
            nc.sync.dma_start(D['x2_dbg'][PT * t:PT * (t + 1)], x2t)
        h2 = layernorm_mod(x2t, sw2b, sh2b, "ht")
        for k in range(KD):
            nc.sync.dma_start_transpose(
                h2T[:, TOK * k + PT * t:TOK * k + PT * (t + 1)],
                h2[:, PT * k:PT * (k + 1)])

    # ---- phase 5: MLP --------------------------------------------------
    mT = persist.tile([PT, KH * TOK], BF, tag="bigA")
    for m in range(KH):
        w1s = stream.tile([PT, KD * PT], BF, tag="w1s")
        for k in range(KD):
            nc.sync.dma_start(w1s[:, PT * k:PT * (k + 1)],
                              D['w1T'][k][:, PT * m:PT * (m + 1)])
        pm = pmm.tile([PT, 512], FP32, tag="pmm")
        for k in range(KD):
            nc.tensor.matmul(
                pm[:, 0:TOK],
                w1s[:, PT * k:PT * (k + 1)],
                h2T[:, TOK * k:TOK * (k + 1)],
                start=(k == 0), stop=(k == KD - 1))
        nc.scalar.activation(mT[:, TOK * m:TOK * (m + 1)], pm[:, 0:TOK],
                             AF.Gelu_apprx_tanh, bias=b1_sb[:, m:m + 1])
    for n in range(2):
        for tp in range(2):
            p2s = [pmm.tile([PT, 512], FP32, tag="pmm", name=f"p2_{n}_{tp}_{i}")
                   for i in range(2)]
            for m in range(KH):
                w2s = stream.tile([PT, 384], BF, tag="w2s")
                nc.sync.dma_start(w2s[:],
                                  D['w2T'][m][:, 384 * n:384 * (n + 1)])
                for i in range(2):
                    t = 2 * tp + i
                    nc.tensor.matmul(
                        p2s[i][:, 0:384],
                        mT[:, TOK * m + PT * t:TOK * m + PT * (t + 1)],
                        w2s[:],
                        start=(m == 0), stop=(m == KH - 1))
            for i in range(2):
                t = 2 * tp + i
                x2t = x2[:, DIM * t:DIM * (t + 1)]
                xg = hpool.tile([PT, 384], FP32, tag="xg")
                nc.gpsimd.tensor_add(xg[:], x2t[:, 384 * n:384 * (n + 1)],
                                     gb2b[:, 384 * n:384 * (n + 1)])
                gm = hpool.tile([PT, 384], FP32, tag="gm")
                nc.vector.tensor_mul(gm[:], p2s[i][:, 0:384],
                                     g2b[:, 384 * n:384 * (n + 1)])
                ot = xpool.tile([PT, 384], FP32, tag="ot")
                nc.vector.tensor_add(ot[:], gm[:], xg[:])
                nc.sync.dma_start(
                    D['out_slice'][PT * t:PT * (t + 1), 384 * n:384 * (n + 1)],
                    ot[:])


def _install_prof_hook():
    """Register the NTFF profile hook concourse expects under axon."""
    import types
    if 'antenv.axon_hooks' in sys.modules:
        return True
    try:
        import antenv
        from trn_agent_boot.trn_boot import _ntff_profile_via_ctypes
        hook = _ntff_profile_via_ctypes('/opt/axon/libaxon_pjrt.so')
        if hook is None:
            return False
        mod = types.ModuleType('antenv.axon_hooks')
        mod.get_axon_ntff_profile_hook = lambda: hook
        mod.set_axon_ntff_profile_hook = lambda h: None
        sys.modules['antenv.axon_hooks'] = mod
        antenv.axon_hooks = mod
        return True
    except Exception as e:
        print(f"prof hook unavailable: {e}")
        return False


def _get_program(taps=False):
    key = ('prog', taps)
    if key not in _CACHE:
        _CACHE[key] = build_program(taps)
    return _CACHE[key]


def kernel(**inputs):
    global LAST_EXEC_NS
    taps = os.environ.get('BASSK_TAPS', '') == '1'
    trace = os.environ.get('BASSK_TRACE', '') == '1'
    nc = _get_program(taps)
    in_maps = host_inputs(**inputs)
    kw = {}
    if trace and _install_prof_hook():
        kw['trace'] = True
    res = run_bass_kernel_spmd(nc, in_maps, core_ids=list(range(NCORES)), **kw)
    LAST_EXEC_NS = res.exec_time_ns
    _CACHE['last_results'] = res
    out = np.empty((B, S, DIM), np.float32)
    for core in range(NCORES):
        b, g = core // GROUP, core % GROUP
        out[b, TOK * g:TOK * (g + 1)] = res.results[core]['out_slice']
    return out
